# revision 2
# baseline (speedup 1.0000x reference)
"""Sliding-window causal GQA attention with ALiBi for Trainium2, SPMD on 8
NeuronCores.

Problem (hardcoded): B=1, S=2048, D=2048, 16 query heads / 4 KV groups,
head_dim 128, window 512.

Sharding: tensor parallel over heads — core c owns KV group c//2 and query
head pair c%2 within that group (2 query heads per core, full sequence).
Wq/Wk/Wv are column-sharded by head, Wo row-sharded; each core produces a
full-shape partial of the output projection and the host sums the 8 partials
(replaces the all-reduce).

Single interleaved PE instruction stream (vs the sequential-phase 207µs
baseline):
- ALiBi+mask applied as exp(s)*T (T = exp(bias) template, zeros where
  masked) via a DVE tensor_tensor multiply, removing ~17K columns of
  identity-matmul bias adds from the PE stream.
- Projections are emitted sequence-chunk-major as lockstep (K,Q0)/(V,Q1)
  psum pairs and woven between attention's scores->exp->mult->PV
  dependency chains as latency padding (depth-3 software pipeline), with
  outproj tiles joining the weave one q-chunk behind attention.  The PE
  rarely waits out the ~1.9µs cross-engine softmax chain.
- All DMA on the two HWDGE rings (sync + scalar) only, ordered by first
  use; xt lands as 64 independent [128,512] tiles so dependency
  granularity is one piece.  The tile framework's DMA-sem optimization
  assumes FIFO completion per HWDGE ring — SWDGE (gpsimd) descriptors and
  DMA-transposes violate that and caused nondeterministic corruption, so
  V is transposed on the PE (psum slots borrowed from the outproj pool)
  and gpsimd issues no DMAs.
- Rowsum reciprocal per head: [1,512] rowsums bounce through DRAM into
  [128,4], one cheap DVE reciprocal, bounce back, stride-0 broadcast
  read; the divide is deferred into a gpsimd multiply so no PE
  instruction waits on the reciprocal chain.
- Tail: the last outproj block runs its head-0 matmuls 4 tiles ahead
  (borrowing idle proj psum slots) so only the head-1 passes wait for the
  final reciprocal chain; stores split across both rings.
"""

import math

import numpy as np
import ml_dtypes

import concourse.bass as bass
import concourse.mybir as mybir
import concourse.tile as tile
from concourse.masks import make_identity

BF16 = ml_dtypes.bfloat16

B, S, D = 1, 2048, 2048
NH, NKV, HD = 16, 4, 128
REP = NH // NKV          # query heads per KV group
WINDOW = 512
NCORES = 8
HPC = 2                  # query heads per core
QC = 512                 # q-chunk width (one PSUM bank of fp32)
NQC = S // QC            # 4
NKT = S // 128           # 16 key tiles
NDC = D // 128           # 16 contraction chunks
TW = WINDOW + 128        # 640: bias template width

FP32 = mybir.dt.float32
BF = mybir.dt.bfloat16


def _alibi_slopes(n_heads: int) -> np.ndarray:
    def pow2_slopes(n):
        start = 2.0 ** (-(2.0 ** (-(math.log2(n) - 3))))
        return [start * start**i for i in range(n)]

    if math.log2(n_heads).is_integer():
        slopes = pow2_slopes(n_heads)
    else:
        closest = 2 ** math.floor(math.log2(n_heads))
        slopes = pow2_slopes(closest)
        slopes += pow2_slopes(2 * closest)[0::2][: n_heads - closest]
    return np.asarray(slopes, dtype=np.float32)


def _exp_templates() -> np.ndarray:
    """[NH, 128, TW] fp32: T[kc, c] = exp(-slope*(c-kc)) where valid
    (0 <= c-kc <= WINDOW-1), else 0.  Template col c of key-tile row kc is
    query position q = k0 + c."""
    slopes = _alibi_slopes(NH)
    kc = np.arange(128)[:, None]
    c = np.arange(TW)[None, :]
    dist = (c - kc).astype(np.float64)
    valid = (dist >= 0) & (dist <= WINDOW - 1)
    out = np.empty((NH, 128, TW), np.float32)
    for h in range(NH):
        out[h] = np.where(valid, np.exp(-float(slopes[h]) * dist), 0.0).astype(
            np.float32
        )
    return out


def _split_waits(nc, maxw=1):
    """This container's walrus rejects instructions with more than one sync
    wait command; hoist extra waits onto preceding same-engine NoOps."""
    plan = {}
    si_type = None
    for bb in nc.main_func.blocks:
        for ins in bb.instructions:
            si = ins.sync_info
            waits = list(si.on_wait) if si and si.on_wait else []
            if len(waits) > maxw:
                si_type = type(si)
                extra = [waits[i:i + maxw] for i in range(0, len(waits) - maxw, maxw)]
                keep = waits[len(extra) * maxw:]
                plan[ins.name] = (extra, keep)
    if not plan:
        return 0
    nops = {}
    nop_names = set()
    for name, (extra, _keep) in plan.items():
        target = nc.inst_map[name]
        eng = nc.engines[target.engine]
        lst = []
        for chunk in extra:
            nop = eng.nop(nofuse=True).ins
            nop.sync_info = si_type(on_wait=chunk, on_update=[])
            lst.append(nop)
            nop_names.add(nop.name)
        nops[name] = lst
    for bb in nc.main_func.blocks:
        insts = list(bb.instructions)
        out = []
        changed = False
        for ins in insts:
            if ins.name in nop_names:
                changed = True
                continue
            if ins.name in plan:
                _extra, keep = plan[ins.name]
                si = ins.sync_info
                upd = list(si.on_update) if si and si.on_update else []
                ins.sync_info = si_type(on_wait=keep, on_update=upd)
                out.extend(nops[ins.name])
                changed = True
            out.append(ins)
        if changed:
            bb.instructions = out
    return len(plan)


def _kts_for(qc):
    """Key tiles feeding q-chunk qc, ordered so the first tile covers the
    full 512 q columns (PSUM has_written needs full coverage on start)."""
    lo = max(0, (qc * QC - WINDOW + 1) // 128)
    hi = (qc * QC + QC - 1) // 128
    if qc == 0:
        return [0, 1, 2, 3]
    # tile 4qc-1 covers q in [q0, q0+511] (w=512) and uses V from chunk
    # qc-1, which was transposed a whole phase earlier - so PV never waits
    # on the fresh chunk's DMA-transposes.
    first = 4 * qc - 1
    return [first] + [t for t in range(lo, hi + 1) if t != first]


def _weave(pads, chains, lead=0):
    """Interleave two lists of emit-closures: emit `lead` pads first (so
    cross-phase copies land before the first chain reads them), then spread
    the rest evenly between chain units."""
    if not chains:
        for p in pads:
            p()
        return
    emitted = 0
    while emitted < min(lead, len(pads)):
        pads[emitted]()
        emitted += 1
    n = len(chains)
    rest = len(pads) - emitted
    base = emitted
    for i, c in enumerate(chains):
        want = base + int(rest * (i + 1) / n)
        c()
        while emitted < want:
            pads[emitted]()
            emitted += 1
    while emitted < len(pads):
        pads[emitted]()
        emitted += 1


def _build_program():
    nc = bass.Bass()

    xt = nc.dram_tensor("xt", [D, S], BF, kind="ExternalInput")
    wq = nc.dram_tensor("wq", [128, HPC * NDC * HD], BF, kind="ExternalInput")
    wk = nc.dram_tensor("wk", [128, NDC * HD], BF, kind="ExternalInput")
    wv = nc.dram_tensor("wv", [128, NDC * HD], BF, kind="ExternalInput")
    wo = nc.dram_tensor("wo", [128, HPC * D], BF, kind="ExternalInput")
    biast = nc.dram_tensor("biast", [128, HPC * TW], BF, kind="ExternalInput")
    out = nc.dram_tensor("out", [S, D], mybir.dt.float16, kind="ExternalOutput")

    with tile.TileContext(nc) as tc:
        with tc.tile_pool(name="persist", bufs=1) as persist:
            # xt as 64 independent tiles: dep granularity = one (dch, sc) piece
            xt_sb = [
                [persist.tile([128, QC], BF, name=f"xt{d}_{s}") for s in range(NQC)]
                for d in range(NDC)
            ]
            # weights split into several tiles: dependency granularity is
            # per-tile, so early matmuls never wait on later weight bytes
            wq_sb = [
                [persist.tile([128, NDC // 2, HD], BF, name=f"wq{h}_{j}")
                 for j in range(2)]
                for h in range(HPC)
            ]
            wk_sb = [persist.tile([128, 4, HD], BF, name=f"wk{j}") for j in range(4)]
            wv_sb = [persist.tile([128, 4, HD], BF, name=f"wv{j}") for j in range(4)]
            wo_sb = [
                [persist.tile([128, D // 2], BF, name=f"wo{h}_{j}") for j in range(2)]
                for h in range(HPC)
            ]
            bias_sb = [persist.tile([128, TW], BF, name=f"bias{h}") for h in range(HPC)]
            qt_sb = [
                [persist.tile([128, QC], BF, name=f"qt{h}_{s}") for s in range(NQC)]
                for h in range(HPC)
            ]
            kt_sb = [persist.tile([128, QC], BF, name=f"kt{i}") for i in range(NQC)]
            vt_sb = [persist.tile([128, QC], BF, name=f"vt{i}") for i in range(NQC)]
            v_sb = [persist.tile([128, HD], BF, name=f"v{i}") for i in range(NKT)]
            yt_sb = [
                [persist.tile([128, QC], BF, name=f"yt{h}_{q}") for q in range(NQC)]
                for h in range(HPC)
            ]
            ones_k = persist.tile([128, 1], BF)
            ident = persist.tile([128, 128], BF)

            nc.vector.memset(ones_k, 1.0)
            make_identity(nc, ident)

            # ---- input DMAs: HWDGE queues only (sync + scalar), ordered
            # by first use.  The tile framework's DMA-sem optimization
            # assumes FIFO completion per HWDGE ring; SWDGE/gpsimd
            # descriptors and DMA-transposes broke that.
            def ld_wk(j):
                nc.sync.dma_start(
                    out=wk_sb[j].rearrange("p c h -> p (c h)"),
                    in_=wk[:, j * 512:(j + 1) * 512])

            def ld_wv(j):
                nc.sync.dma_start(
                    out=wv_sb[j].rearrange("p c h -> p (c h)"),
                    in_=wv[:, j * 512:(j + 1) * 512])

            def ld_wq(h, j):
                nc.scalar.dma_start(
                    out=wq_sb[h][j].rearrange("p c h -> p (c h)"),
                    in_=wq[:, h * NDC * HD + j * 1024:
                           h * NDC * HD + (j + 1) * 1024])

            def ld_xt(dch, sc, eng):
                eng.dma_start(
                    out=xt_sb[dch][sc],
                    in_=xt[dch * 128:(dch + 1) * 128, sc * QC:(sc + 1) * QC])

            # phase-0 need order: K/V sweep consumes (dch, 0) pieces in dch
            # order; weights interleaved just ahead of their chunks
            for j in range(4):
                ld_wk(j)
                ld_wv(j)
                ld_xt(4 * j, 0, nc.sync)
                ld_xt(4 * j + 2, 0, nc.sync)
            for j in range(2):
                ld_xt(8 * j + 1, 0, nc.scalar)
                ld_xt(8 * j + 3, 0, nc.scalar)
                ld_wq(0, j)
                ld_xt(8 * j + 5, 0, nc.scalar)
                ld_xt(8 * j + 7, 0, nc.scalar)
                ld_wq(1, j)
            for h in range(HPC):
                nc.sync.dma_start(out=bias_sb[h],
                                  in_=biast[:, h * TW:(h + 1) * TW])

            def xt_load(sc, queues):
                for dch in range(NDC):
                    ld_xt(dch, sc, queues[dch % len(queues)])

            xt_load(1, [nc.sync])
            for h in range(HPC):
                for j in range(2):
                    nc.sync.dma_start(
                        out=wo_sb[h][j],
                        in_=wo[:, h * D + j * (D // 2):h * D + (j + 1) * (D // 2)])

            with tc.tile_pool(name="proj_ps", bufs=2, space="PSUM") as proj_ps, \
                 tc.tile_pool(name="sc_ps", bufs=2, space="PSUM") as sc_ps, \
                 tc.tile_pool(name="y_ps", bufs=1, space="PSUM") as y_ps_pool, \
                 tc.tile_pool(name="r_ps", bufs=1, space="PSUM") as r_ps_pool, \
                 tc.tile_pool(name="op_ps", bufs=2, space="PSUM") as op_ps, \
                 tc.tile_pool(name="et_sb", bufs=5) as et_pool, \
                 tc.tile_pool(name="small_sb", bufs=6) as small, \
                 tc.tile_pool(name="rc_sb", bufs=2) as rc_pool, \
                 tc.tile_pool(name="stg_sb", bufs=3) as stg_pool, \
                 tc.tile_pool(name="r_dram", bufs=4, space="DRAM") as r_dram:

                # ---------- projection pads (sc-major) ----------
                # One projection at a time, units of 2 contraction steps.
                # `pool` alternates proj_ps/op_ps in early phases (op pool
                # is idle there) so back-to-back projections don't stall
                # on the copy-out of a single buffer.
                def proj_pair_units(sc, pair):
                    """Lockstep pair per dch unit (two psums accumulate
                    side by side, exactly the v2-proven pattern).  Pair 0 =
                    (K, V) so V's DMA-transposes fire mid-phase; pair 1 =
                    (Q0, Q1)."""
                    units = []
                    hold = {}

                    def mk(dch):
                        def u():
                            if dch == 0:
                                hold['a'] = proj_ps.tile(
                                    [128, QC], FP32, tag="proj", name="proj_a")
                                hold['b'] = proj_ps.tile(
                                    [128, QC], FP32, tag="proj", name="proj_b")
                            first, last = dch == 0, dch == NDC - 1
                            if pair == 0:
                                lhsA = wk_sb[dch // 4][:, dch % 4, :]
                                lhsB = wv_sb[dch // 4][:, dch % 4, :]
                            else:
                                lhsA = wq_sb[0][dch // 8][:, dch % 8, :]
                                lhsB = wq_sb[1][dch // 8][:, dch % 8, :]
                            nc.tensor.matmul(
                                hold['a'], lhsA, xt_sb[dch][sc],
                                start=first, stop=last,
                            )
                            nc.tensor.matmul(
                                hold['b'], lhsB, xt_sb[dch][sc],
                                start=first, stop=last,
                            )
                            if last:
                                if pair == 0:
                                    nc.scalar.copy(out=kt_sb[sc], in_=hold['a'])
                                    nc.scalar.copy(out=vt_sb[sc], in_=hold['b'])
                                else:
                                    nc.scalar.copy(out=qt_sb[0][sc], in_=hold['a'])
                                    nc.scalar.copy(out=qt_sb[1][sc], in_=hold['b'])
                        return u

                    for dch in range(NDC):
                        units.append(mk(dch))
                    return units

                def vt_units(sc):
                    # PE transposes of the V chunk, borrowing op-pool slots
                    units = []

                    def mk(j):
                        def u():
                            kt = sc * 4 + j
                            tp = op_ps.tile([128, 128], BF, tag="op", name="tp")
                            nc.tensor.transpose(
                                tp, vt_sb[sc][:, j * 128:(j + 1) * 128], ident)
                            nc.scalar.copy(out=v_sb[kt], in_=tp)
                        return u

                    for j in range(4):
                        units.append(mk(j))
                    return units

                def proj_phase_units(sc, alternate):
                    p1 = proj_pair_units(sc, 1)
                    tps = vt_units(sc)
                    # weave the 4 transposes into pair-1's early units so the
                    # vt copy (scalar) has time to land first
                    merged = []
                    for i, u in enumerate(p1):
                        merged.append(u)
                        if 1 <= i <= 4:
                            merged.append(tps[i - 1])
                    return proj_pair_units(sc, 0) + merged

                # ---------- attention chain units ----------
                def attention_units(h, qc, qshare):
                    """Software-pipelined: unit i emits S(t_i); PV/R of
                    t_{i-DEPTH} ride unit i so the exp+mult chain hides
                    behind woven pads.  qshare carries the per-qc merged
                    rowsum buffer: one DRAM bounce + one reciprocal serves
                    both heads."""
                    q0 = qc * QC
                    kts = _kts_for(qc)
                    n = len(kts)
                    st = {}

                    def geom(kt):
                        k0 = kt * 128
                        q_lo = max(q0, k0)
                        q_hi = min(q0 + QC - 1, k0 + TW - 1)
                        return k0, q_lo, q_hi - q_lo + 1

                    def emit_S(i):
                        kt = kts[i]
                        k0, q_lo, w = geom(kt)
                        s_ps = sc_ps.tile([128, QC], FP32, tag="sc", name="s_ps")
                        nc.tensor.matmul(
                            s_ps[:, :w],
                            kt_sb[kt // 4][:, (kt % 4) * 128:(kt % 4) * 128 + 128],
                            qt_sb[h][qc][:, q_lo - q0:q_lo - q0 + w],
                            start=True, stop=True,
                        )
                        et0 = et_pool.tile([128, QC], BF, tag="et0", name="et0")
                        nc.scalar.activation(
                            out=et0[:, :w], in_=s_ps[:, :w],
                            func=mybir.ActivationFunctionType.Exp,
                        )
                        et = et_pool.tile([128, QC], BF, tag="et", name="et")
                        nc.vector.tensor_tensor(
                            et[:, :w], et0[:, :w],
                            bias_sb[h][:, q_lo - k0:q_lo - k0 + w],
                            mybir.AluOpType.mult,
                        )
                        st[i] = (et, kt)

                    def emit_PVR(i):
                        et, kt = st.pop(i)
                        k0, q_lo, w = geom(kt)
                        first, last = i == 0, i == n - 1
                        nc.tensor.matmul(
                            st['y'][:, q_lo - q0:q_lo - q0 + w],
                            v_sb[kt], et[:, :w],
                            start=first, stop=last, skip_group_check=True,
                        )
                        nc.tensor.matmul(
                            st['r'][:, q_lo - q0:q_lo - q0 + w],
                            ones_k, et[:, :w],
                            start=first, stop=last, skip_group_check=True,
                        )

                    def finish():
                        # per-head chain: h0's bounce overlaps A(1,qc)'s
                        # whole compute. 4 FIFO-ordered hops on one HWDGE
                        # queue + a tiny [128,4] DVE reciprocal.
                        eng = nc.scalar if qc < 2 else nc.sync
                        yun = small.tile([128, QC], FP32, tag="yun", name="yun")
                        nc.vector.tensor_copy(yun, st['y'])
                        r_sb = small.tile([1, QC], FP32, tag="rsb", name="r_sb")
                        nc.scalar.copy(out=r_sb, in_=st['r'])
                        rd1 = r_dram.tile([1, QC], FP32, tag="rd1", name="rd1")
                        eng.dma_start(out=rd1, in_=r_sb)
                        r128 = rc_pool.tile([128, 4], FP32, tag="r128", name="r128")
                        srcap = bass.AP(tensor=rd1.tensor, offset=rd1.offset,
                                        ap=[[4, 128], [1, 4]])
                        eng.dma_start(out=r128, in_=srcap)
                        rc2 = rc_pool.tile([128, 4], FP32, tag="rc2", name="rc2")
                        nc.vector.reciprocal(rc2, r128)
                        rd2 = r_dram.tile([1, QC], FP32, tag="rd2", name="rd2")
                        dstap = bass.AP(tensor=rd2.tensor, offset=rd2.offset,
                                        ap=[[4, 128], [1, 4]])
                        eng.dma_start(out=dstap, in_=rc2)
                        rb = small.tile([128, QC], FP32, tag="rb", name="rb")
                        bc = bass.AP(tensor=rd2.tensor, offset=rd2.offset,
                                     ap=[[0, 128], [1, QC]])
                        eng.dma_start(out=rb, in_=bc)
                        nc.gpsimd.tensor_tensor(
                            yt_sb[h][qc], yun, rb, mybir.AluOpType.mult
                        )

                    units = []
                    DEPTH = 3

                    def mk(i):
                        def u():
                            if i == 0:
                                st['y'] = y_ps_pool.tile([128, QC], FP32, tag="y", name="y_ps")
                                st['r'] = r_ps_pool.tile([1, QC], FP32, tag="r", name="r_ps")
                            if i < n:
                                emit_S(i)
                            if i >= DEPTH:
                                emit_PVR(i - DEPTH)
                            if i == n + DEPTH - 1:
                                finish()
                        return u

                    for i in range(n + DEPTH):
                        units.append(mk(i))
                    return units

                # ---------- outproj pads ----------
                def outproj_units(qc, runahead=False):
                    """16 tiles of 2 matmuls (head accumulate); staging
                    copies mostly on DVE; merged stores.  With runahead, the
                    h0 matmuls run 4 tiles ahead (borrowing the idle proj
                    psum slots) so the tail only waits on yt[1]'s chain for
                    the h1 passes."""
                    units = []
                    st = {}
                    depth = 4 if runahead else 1
                    pools = [op_ps, op_ps, proj_ps, proj_ps]
                    tags = ["op", "op", "proj", "proj"]

                    def start_tile(t):
                        sti, ncol = divmod(t, 4)
                        pool = pools[t % depth] if runahead else op_ps
                        tag = tags[t % depth] if runahead else "op"
                        ps = pool.tile([128, QC], FP32, tag=tag, name="op_t")
                        st[t] = ps
                        nc.tensor.matmul(
                            ps,
                            yt_sb[0][qc][:, sti * 128:(sti + 1) * 128],
                            wo_sb[0][ncol // 2][
                                :, (ncol % 2) * QC:(ncol % 2) * QC + QC],
                            start=True, stop=False,
                        )

                    def finish_tile(t):
                        sti, ncol = divmod(t, 4)
                        ps = st.pop(t)
                        nc.tensor.matmul(
                            ps,
                            yt_sb[1][qc][:, sti * 128:(sti + 1) * 128],
                            wo_sb[1][ncol // 2][
                                :, (ncol % 2) * QC:(ncol % 2) * QC + QC],
                            start=False, stop=True,
                        )
                        if ncol == 0:
                            st['stg'] = stg_pool.tile(
                                [128, D], mybir.dt.float16, tag="stg",
                                name="stg")
                        stg = st['stg']
                        dst = stg[:, ncol * QC:(ncol + 1) * QC]
                        if ncol == 0:
                            nc.scalar.copy(out=dst, in_=ps)
                        else:
                            nc.vector.tensor_copy(dst, ps)
                        if ncol == 3:
                            srow = qc * 4 + sti
                            if runahead:
                                for j in range(4):
                                    eng = nc.sync if j % 2 == 0 else nc.scalar
                                    eng.dma_start(
                                        out=out[srow * 128:(srow + 1) * 128,
                                                j * QC:(j + 1) * QC],
                                        in_=stg[:, j * QC:(j + 1) * QC],
                                    )
                            else:
                                nc.sync.dma_start(
                                    out=out[srow * 128:(srow + 1) * 128, :D // 2],
                                    in_=stg[:, :D // 2],
                                )
                                nc.scalar.dma_start(
                                    out=out[srow * 128:(srow + 1) * 128, D // 2:],
                                    in_=stg[:, D // 2:],
                                )

                    def mk(i):
                        def u():
                            if i == 0:
                                for t in range(depth):
                                    start_tile(t)
                            finish_tile(i)
                            if i + depth < 16:
                                start_tile(i + depth)
                        return u

                    for i in range(16):
                        units.append(mk(i))
                    return units

                # ---------- the weave ----------
                # warm the activation table before the DMA prologue ends
                warm = small.tile([128, 1], FP32, tag="warm", name="warm")
                nc.scalar.activation(out=warm, in_=ones_k,
                                     func=mybir.ActivationFunctionType.Exp)

                # phase 0: proj sc0 (no chains yet, DMA-limited anyway)
                for u in proj_phase_units(0, alternate=False):
                    u()
                qsh = [{} for _ in range(NQC)]
                xt_load(2, [nc.sync])
                # phase 1: proj sc1  x  attention qc0
                _weave(
                    proj_phase_units(1, alternate=False),
                    attention_units(0, 0, qsh[0]) + attention_units(1, 0, qsh[0]),
                    lead=4,
                )
                xt_load(3, [nc.sync])
                # phase 2: proj sc2 + op0  x  attention qc1
                _weave(
                    proj_phase_units(2, alternate=False) + outproj_units(0),
                    attention_units(0, 1, qsh[1]) + attention_units(1, 1, qsh[1]),
                    lead=6,
                )
                # phase 3: proj sc3 + op1(part)  x  attention qc2
                _weave(
                    proj_phase_units(3, alternate=False) + outproj_units(1),
                    attention_units(0, 2, qsh[2]) + attention_units(1, 2, qsh[2]),
                    lead=6,
                )
                # phase 4: op2  x  attention qc3, chains front-loaded so the
                # last recip chain launches as early as possible
                ph4_pads = outproj_units(2)
                ph4_chains = (attention_units(0, 3, qsh[3])
                              + attention_units(1, 3, qsh[3]))
                _weave(ph4_pads, ph4_chains, lead=2)
                # tail: op3 with h0 run-ahead
                for u in outproj_units(3, runahead=True):
                    u()

    _split_waits(nc, maxw=1)
    return nc


_NC_CACHE = None


def _get_program():
    global _NC_CACHE
    if _NC_CACHE is None:
        _NC_CACHE = _build_program()
    return _NC_CACHE


def _shuffle_chunks(w, cols):
    """[D, cols] -> [128, NDC*cols] partition-major contiguous layout."""
    return np.ascontiguousarray(
        w.reshape(NDC, 128, cols).transpose(1, 0, 2).reshape(128, NDC * cols)
    )


def build_in_maps(x, Wq, Wk, Wv, Wo):
    x = np.asarray(x, np.float32)
    Wq = np.asarray(Wq, np.float32)
    Wk = np.asarray(Wk, np.float32)
    Wv = np.asarray(Wv, np.float32)
    Wo = np.asarray(Wo, np.float32)

    xt = np.ascontiguousarray(x[0].T).astype(BF16)
    wq_s = (Wq * (1.0 / math.sqrt(HD))).astype(BF16)
    wk_s = Wk.astype(BF16)
    wv_s = Wv.astype(BF16)
    wo_s = Wo.astype(BF16)
    templates = _exp_templates()

    in_maps = []
    for c in range(NCORES):
        g, hp = c // HPC, c % HPC
        heads = [g * REP + hp * HPC + r for r in range(HPC)]
        # wq laid out [128, h, dch, HD]
        wq_heads = np.concatenate(
            [_shuffle_chunks(wq_s[:, h * HD:(h + 1) * HD], HD) for h in heads],
            axis=1,
        )
        wo_rows = wo_s[heads[0] * HD:(heads[-1] + 1) * HD, :]  # [256, D]
        in_maps.append(
            {
                "xt": xt,
                "wq": wq_heads,
                "wk": _shuffle_chunks(wk_s[:, g * HD:(g + 1) * HD], HD),
                "wv": _shuffle_chunks(wv_s[:, g * HD:(g + 1) * HD], HD),
                "wo": np.ascontiguousarray(
                    wo_rows.reshape(HPC, 128, D).transpose(1, 0, 2).reshape(128, HPC * D)
                ),
                "biast": np.ascontiguousarray(
                    templates[heads].transpose(1, 0, 2).reshape(128, HPC * TW)
                ).astype(BF16),
            }
        )
    return in_maps


_last_in_maps = None


def kernel(x, Wq, Wk, Wv, Wo):
    from concourse.bass_utils import run_bass_kernel_spmd

    global _last_in_maps
    in_maps = build_in_maps(x, Wq, Wk, Wv, Wo)
    _last_in_maps = in_maps

    nc = _get_program()
    res = run_bass_kernel_spmd(nc, in_maps, list(range(NCORES)))
    acc = res.results[0]["out"].astype(np.float64)
    for c in range(1, NCORES):
        acc += res.results[c]["out"]
    return acc.astype(np.float32).reshape(B, S, D)
